# revision 17
# baseline (speedup 1.0000x reference)
"""Trainium2 Bass kernel for nn_PolicyGNN_v3 (3-layer GATv2 GNN, 20000 nodes,
320000 edges + self loops), sharded across 8 NeuronCores.

Sharding: nodes partitioned contiguously across cores (2500/core); each edge is
processed by the core owning its dst node. Per layer, each core projects its
node shard (xl = h @ Wl), the shards are AllGathered into a full per-node
message table, and the edge phase gathers per-edge rows, computes GATv2
attention with a segment-local softmax (all edges of a dst node are packed into
one 128-edge batch), and scatter-adds per-node results back to HBM.

Self-contained: all shapes/graph preprocessing hardcoded/derived from inputs.
"""
import numpy as np
import ml_dtypes

N = 20000
E = 320000
DIN = 64
C = 256
HEADS = 4
NCORES = 8
NPC = N // NCORES          # 2500 nodes per core
SEG_MAX = 32               # segment slots per 128-edge batch
GB = 4                     # batches per gather/scatter group
BATCH_E = 128
NEG_ATT = 0.2
NEG = 0.01
PHASES = 99                # debug: build only stages <= PHASES
LOOP_K = 1                 # timing: repeat whole body K times (hoists AllGathers)

BF = ml_dtypes.bfloat16

# ---------------------------------------------------------------------------
# host-side graph preprocessing
# ---------------------------------------------------------------------------


def _preprocess(src, dst):
    """Split edges (plus self loops) by dst-owner core, sort by dst, pack into
    128-edge batches of whole dst-segments (<=SEG_MAX segments per batch).

    Returns per-core dict of device-layout index arrays, padded to the max
    batch count over cores (rounded up to a multiple of GB)."""
    s = np.concatenate([src, np.arange(N, dtype=np.int64)])
    d = np.concatenate([dst, np.arange(N, dtype=np.int64)])
    order = np.argsort(d, kind="stable")
    s, d = s[order], d[order]
    deg = np.bincount(d, minlength=N)
    assert deg.max() <= BATCH_E
    starts = np.concatenate([[0], np.cumsum(deg)])

    per_core = []
    for c in range(NCORES):
        v0, v1 = c * NPC, (c + 1) * NPC
        batches = []          # (src_ids, dst_loc, P, segid)
        cur_edges = []        # (src, dst_local)
        cur_segs = []         # (v_local, deg)

        def close():
            nonlocal cur_edges, cur_segs
            if not cur_segs:
                return
            src_ids = np.zeros(BATCH_E, np.int16)
            dst_loc = np.full(BATCH_E, NPC, np.int16)
            P = np.zeros((BATCH_E, SEG_MAX), np.float32)
            segid = np.full(SEG_MAX, NPC, np.int16)
            k = 0
            for si, (vloc, dg) in enumerate(cur_segs):
                segid[si] = vloc
                for _ in range(dg):
                    src_ids[k], dst_loc[k] = cur_edges[k]
                    P[k, si] = 1.0
                    k += 1
            batches.append((src_ids, dst_loc, P, segid))
            cur_edges, cur_segs = [], []

        for v in range(v0, v1):
            dg = int(deg[v])
            if len(cur_edges) + dg > BATCH_E or len(cur_segs) >= SEG_MAX:
                close()
            st = starts[v]
            for i in range(dg):
                cur_edges.append((int(s[st + i]), v - v0))
            cur_segs.append((v - v0, dg))
        close()
        per_core.append(batches)

    nb_max = max(len(b) for b in per_core)
    NB = ((nb_max + GB - 1) // GB) * GB
    pad_batch = (np.zeros(BATCH_E, np.int16),
                 np.full(BATCH_E, NPC, np.int16),
                 np.zeros((BATCH_E, SEG_MAX), np.float32),
                 np.full(SEG_MAX, NPC, np.int16))
    out = []
    for batches in per_core:
        batches = batches + [pad_batch] * (NB - len(batches))
        all_src = np.concatenate([b[0] for b in batches])            # [NB*128]
        all_dst = np.concatenate([b[1] for b in batches])
        all_seg = np.concatenate([b[3] for b in batches])            # [NB*16]
        pmat = np.stack([b[2] for b in batches])                     # [NB,128,16]
        # gather idx layout: element i of a 512-idx group at [i%16, 32g+i//16],
        # globally = reshape(-1,16).T ; replicated x8 on partition axis
        al16 = np.tile(all_src.reshape(-1, 16).T, (8, 1)).astype(np.int16)
        ar16 = np.tile(all_dst.reshape(-1, 16).T, (8, 1)).astype(np.int16)
        sc16 = np.tile(all_seg.reshape(-1, 16).T, (8, 1)).astype(np.int16)
        # pmat device layout [128, NB*16] (batch b -> cols 16b:16b+16)
        pm = np.ascontiguousarray(pmat.transpose(1, 0, 2).reshape(128, NB * SEG_MAX)).astype(BF)
        out.append(dict(al16=al16, ar16=ar16, sc16=sc16, pmat=pm))
    return out, NB


# ---------------------------------------------------------------------------
# bass program
# ---------------------------------------------------------------------------

def _build(NB, trn_type="TRN2"):
    import concourse.bass as bass
    import concourse.mybir as mybir
    import concourse.tile as tile
    import concourse.bacc as bacc

    f32 = mybir.dt.float32
    bf16 = mybir.dt.bfloat16
    i16 = mybir.dt.int16
    AX = mybir.AxisListType.X
    OP = mybir.AluOpType
    ACT = mybir.ActivationFunctionType

    NG = NB // GB
    NT = (NPC + 127) // 128          # 20 node tiles
    RN = [min(128, NPC - t * 128) for t in range(NT)]
    NTRAW = (NPC + 1 + 127) // 128   # tiles to zero h_raw (incl trash row)

    nc = bacc.Bacc(trn_type, target_bir_lowering=False, debug=False,
                   enable_asserts=False, num_devices=NCORES)

    def din(name, shape, dt):
        return nc.dram_tensor(name, shape, dt, kind="ExternalInput").ap()

    # per-core data
    x_loc = din("x_loc", [NPC, DIN], bf16)
    al16 = din("al16", [128, NB * 8], i16)
    ar16 = din("ar16", [128, NB * 8], i16)
    sc16 = din("sc16", [128, NB * SEG_MAX // 16], i16)
    pmat = din("pmat", [128, NB * SEG_MAX], bf16)
    # weights (replicated)
    w_in = din("w_in", [DIN, C], bf16)
    b_in = din("b_in", [1, C], bf16)
    WL = [din(f"wl{l}", [C if l == 1 else HEADS * C, HEADS * C if l < 3 else C], bf16)
          for l in (1, 2, 3)]
    WR = [din(f"wr{l}", [C if l == 1 else HEADS * C, HEADS * C if l < 3 else C], bf16)
          for l in (1, 2, 3)]
    BL = [din(f"bl{l}", [1, HEADS * C if l < 3 else C], bf16) for l in (1, 2, 3)]
    BR = [din(f"br{l}", [1, HEADS * C if l < 3 else C], bf16) for l in (1, 2, 3)]
    ATTR = [din(f"attrep{l}", [128, HEADS * C if l < 3 else C], bf16) for l in (1, 2, 3)]
    BIASR = [din(f"biasrep{l}", [128, HEADS * C if l < 3 else C], f32) for l in (1, 2, 3)]
    woutrep = din("woutrep", [128, C], f32)
    boutrep = din("boutrep", [128, 1], f32)
    identbf = din("identbf", [128, 128], bf16)
    onesbf = din("onesbf", [1, 128], bf16)

    logits = nc.dram_tensor("logits", [NPC], f32, kind="ExternalOutput").ap()

    with tile.TileContext(nc) as tc:
        import contextlib
        stack = contextlib.ExitStack()
        with stack:
            dpool = stack.enter_context(tc.tile_pool(name="dram", bufs=1, space="DRAM"))
            cpool = stack.enter_context(tc.tile_pool(name="const", bufs=1))

            # ---------------- DRAM scratch ----------------
            hT0 = dpool.tile([2, NT, 128, 128], bf16)          # h_local transposed, blocked
            h_loc = dpool.tile([NPC, C], bf16)
            layer_cfg = []
            for li, l in enumerate((1, 2, 3)):
                d_in = C if l == 1 else HEADS * C
                heads = HEADS if l < 3 else 1
                cdim = C
                d_out = heads * cdim
                raw_w = d_out + 64                              # +den cols (padded)
                cc_in = dpool.tile([NPC, d_out], bf16, name=f"ccin{l}")
                cc_out = dpool.tile([N, d_out], bf16, addr_space="Shared", name=f"ccout{l}")
                xr = dpool.tile([NPC + 1, d_out], bf16, name=f"xr{l}")
                hraw = dpool.tile([NPC + 1, raw_w], f32, name=f"hraw{l}")
                hT = (dpool.tile([d_out // 128, NT, 128, 128], bf16, name=f"hT{l}")
                      if l < 3 else None)
                layer_cfg.append(dict(l=l, d_in=d_in, heads=heads, cdim=cdim,
                                      d_out=d_out, raw_w=raw_w, cc_in=cc_in,
                                      cc_out=cc_out, xr=xr, hraw=hraw, hT=hT))

            # ---------------- resident constants ----------------
            ident_sb = cpool.tile([128, 128], bf16)
            nc.sync.dma_start(out=ident_sb[:], in_=identbf[:])
            ones_sb = cpool.tile([1, 128], bf16)
            nc.sync.dma_start(out=ones_sb[:], in_=onesbf[:])
            al_sb = cpool.tile([128, NB * 8], i16)
            nc.sync.dma_start(out=al_sb[:], in_=al16[:])
            ar_sb = cpool.tile([128, NB * 8], i16)
            nc.sync.dma_start(out=ar_sb[:], in_=ar16[:])
            sc_sb = cpool.tile([128, NB * SEG_MAX // 16], i16)
            nc.sync.dma_start(out=sc_sb[:], in_=sc16[:])
            pm_sb = cpool.tile([128, NB * SEG_MAX], bf16)
            nc.sync.dma_start(out=pm_sb[:], in_=pmat[:])
            attr_sb = []
            biasr_sb = []
            for li in range(3):
                a = cpool.tile([128, layer_cfg[li]["d_out"]], bf16, name=f"attr_sb{li}")
                nc.sync.dma_start(out=a[:], in_=ATTR[li][:])
                attr_sb.append(a)
                b = cpool.tile([128, layer_cfg[li]["d_out"]], f32, name=f"biasr_sb{li}")
                nc.sync.dma_start(out=b[:], in_=BIASR[li][:])
                biasr_sb.append(b)
            wout_sb = cpool.tile([128, C], f32)
            nc.sync.dma_start(out=wout_sb[:], in_=woutrep[:])
            bout_sb = cpool.tile([128, 1], f32)
            nc.sync.dma_start(out=bout_sb[:], in_=boutrep[:])
            zero_sb = cpool.tile([128, 1088], f32)
            nc.vector.memset(zero_sb[:], 0.0)
            zero_bf = cpool.tile([1, 1024], bf16)
            nc.vector.memset(zero_bf[:], 0.0)
            logit_sb = cpool.tile([128, NT], f32)

            # For timing amplification: run AllGathers once up front (they
            # cannot sit inside control flow), then loop the whole body.
            if LOOP_K > 1:
                for cfg in layer_cfg:
                    nc.gpsimd.collective_compute(
                        "AllGather", OP.bypass,
                        replica_groups=[list(range(NCORES))],
                        ins=[cfg["cc_in"][:].opt()],
                        outs=[cfg["cc_out"][:].opt()],
                    )
                loopctx = tc.For_i(0, LOOP_K, 1)
            else:
                loopctx = contextlib.nullcontext()
            loopctx.__enter__()

            # ---------------- input layer ----------------
            if PHASES >= 1:
             with tc.tile_pool(name="inp", bufs=3) as ip, \
                 tc.tile_pool(name="inpps", bufs=2, space="PSUM") as ips:
                win_sb = ip.tile([DIN, C], bf16, bufs=1)
                nc.sync.dma_start(out=win_sb[:], in_=w_in[:])
                bin_sb = ip.tile([1, C], bf16, bufs=1)
                nc.sync.dma_start(out=bin_sb[:], in_=b_in[:])
                for t in range(NT):
                    rn, r0 = RN[t], t * 128
                    xt = ip.tile([128, DIN], bf16)
                    nc.sync.dma_start(out=xt[:rn, :], in_=x_loc[r0:r0 + rn, :])
                    xT_ps = ips.tile([DIN, 128], bf16, space="PSUM")
                    nc.tensor.transpose(out=xT_ps[:, :rn], in_=xt[:rn, :],
                                        identity=ident_sb[:rn, :rn])
                    xT = ip.tile([DIN, 128], bf16)
                    nc.vector.tensor_copy(out=xT[:, :rn], in_=xT_ps[:, :rn])
                    ps = ips.tile([128, C], f32, space="PSUM")
                    nc.tensor.matmul(ps[:rn, :], lhsT=ones_sb[:, :rn], rhs=bin_sb[:],
                                     start=True, stop=False)
                    nc.tensor.matmul(ps[:rn, :], lhsT=xT[:, :rn], rhs=win_sb[:],
                                     start=False, stop=True)
                    hsb = ip.tile([128, C], f32)
                    nc.scalar.copy(out=hsb[:rn, :], in_=ps[:rn, :])
                    hb = ip.tile([128, C], bf16)
                    nc.vector.scalar_tensor_tensor(out=hb[:rn, :], in0=hsb[:rn, :],
                                                   scalar=NEG, op0=OP.mult,
                                                   in1=hsb[:rn, :], op1=OP.max)
                    nc.sync.dma_start(out=h_loc[r0:r0 + rn, :], in_=hb[:rn, :])
                    psT = ips.tile([128, 2, 128], bf16, space="PSUM")
                    for f in range(2):
                        nc.tensor.transpose(out=psT[:, f, :rn], in_=hb[:rn, f * 128:(f + 1) * 128],
                                            identity=ident_sb[:rn, :rn])
                    hTsb = ip.tile([128, 2, 128], bf16)
                    nc.vector.tensor_copy(out=hTsb[:, :, :rn], in_=psT[:, :, :rn])
                    nc.sync.dma_start(out=hT0[:, t, :, :rn].rearrange("k p n -> p k n"),
                                      in_=hTsb[:, :, :rn])

            # ---------------- GAT layers ----------------
            prev_hT = hT0
            for li, cfg in enumerate(layer_cfg):
                base = 4 * li + 2        # proj, ag, edge, epi stages
                l, d_in, heads, cdim = cfg["l"], cfg["d_in"], cfg["heads"], cfg["cdim"]
                d_out, raw_w = cfg["d_out"], cfg["raw_w"]
                KCH = d_in // 128
                NCH = (d_out + 511) // 512
                nw_full = d_out // NCH                       # 512 or 256
                den_col = d_out                              # den starts here in psum/hraw

                # ---- projections + zero hraw ----
                if PHASES < base:
                    break
                with tc.tile_pool(name=f"proj{l}", bufs=3) as pp, \
                     tc.tile_pool(name=f"projw{l}", bufs=1) as pw, \
                     tc.tile_pool(name=f"projps{l}", bufs=4, space="PSUM") as pps:
                    wsb = {}
                    for side, W, B in (("l", WL[li], BL[li]), ("r", WR[li], BR[li])):
                        for k in range(KCH):
                            for cch in range(NCH):
                                w = pw.tile([128, nw_full], bf16, name=f"w{l}{side}{k}{cch}")
                                nc.sync.dma_start(
                                    out=w[:],
                                    in_=W[k * 128:(k + 1) * 128,
                                          cch * nw_full:(cch + 1) * nw_full])
                                wsb[(side, k, cch)] = w
                        bsb = pw.tile([1, d_out], bf16, name=f"b{l}{side}")
                        nc.sync.dma_start(out=bsb[:], in_=(B[:]))
                        wsb[(side, "bias")] = bsb
                    # zero hraw (+ trash row) while weights load
                    for t in range(NTRAW):
                        rn0 = min(128, NPC + 1 - t * 128)
                        nc.sync.dma_start(out=cfg["hraw"][t * 128:t * 128 + rn0, :],
                                          in_=zero_sb[:rn0, :raw_w])
                    # zero xr trash row
                    nc.sync.dma_start(out=cfg["xr"][NPC:NPC + 1, :], in_=zero_bf[:, :d_out])

                    for t in range(NT):
                        rn, r0 = RN[t], t * 128
                        hts = []
                        for k in range(KCH):
                            ht = pp.tile([128, 128], bf16, tag=f"ht{k}", bufs=2)
                            nc.sync.dma_start(out=ht[:, :rn], in_=prev_hT[k, t, :, :rn])
                            hts.append(ht)
                        for side, dest in (("l", cfg["cc_in"]), ("r", cfg["xr"])):
                            for cch in range(NCH):
                                nw = nw_full
                                ps = pps.tile([128, nw_full], f32, space="PSUM", tag="pps")
                                nc.tensor.matmul(
                                    ps[:rn, :nw], lhsT=ones_sb[:, :rn],
                                    rhs=wsb[(side, "bias")][:, cch * nw:(cch + 1) * nw],
                                    start=True, stop=False)
                                for k in range(KCH):
                                    nc.tensor.matmul(ps[:rn, :nw], lhsT=hts[k][:, :rn],
                                                     rhs=wsb[(side, k, cch)][:],
                                                     start=False, stop=(k == KCH - 1))
                                ob = pp.tile([128, nw_full], bf16, tag="ob")
                                nc.scalar.copy(out=ob[:rn, :nw], in_=ps[:rn, :nw])
                                nc.sync.dma_start(
                                    out=dest[r0:r0 + rn, cch * nw:(cch + 1) * nw],
                                    in_=ob[:rn, :nw])

                # ---- AllGather xl shards ----
                if PHASES < base + 1:
                    break
                if LOOP_K == 1:
                    nc.gpsimd.collective_compute(
                        "AllGather", OP.bypass,
                        replica_groups=[list(range(NCORES))],
                        ins=[cfg["cc_in"][:].opt()],
                        outs=[cfg["cc_out"][:].opt()],
                    )

                # ---- edge phase ----
                if PHASES < base + 2:
                    break
                with tc.tile_pool(name=f"edge{l}", bufs=2) as ep, \
                     tc.tile_pool(name=f"edgeps{l}", bufs=2, space="PSUM") as eps:
                    for g in range(NG):
                        AL = ep.tile([128, GB, d_out], bf16, tag="AL")
                        nc.gpsimd.dma_gather(
                            out_ap=AL[:], in_ap=cfg["cc_out"][:],
                            idxs_ap=al_sb[:, 32 * g:32 * g + 32],
                            num_idxs=GB * 128, num_idxs_reg=GB * 128,
                            elem_size=d_out)
                        AR = ep.tile([128, GB, d_out], bf16, tag="AR")
                        nc.gpsimd.dma_gather(
                            out_ap=AR[:], in_ap=cfg["xr"][:],
                            idxs_ap=ar_sb[:, 32 * g:32 * g + 32],
                            num_idxs=GB * 128, num_idxs_reg=GB * 128,
                            elem_size=d_out)
                        Z = ep.tile([128, GB, d_out], bf16, tag="Z")
                        nc.vector.tensor_add(out=Z[:], in0=AL[:], in1=AR[:])
                        Lx = ep.tile([128, GB, d_out], bf16, tag="Lx")
                        nc.vector.scalar_tensor_tensor(out=Lx[:], in0=Z[:], scalar=NEG_ATT,
                                                       op0=OP.mult, in1=Z[:], op1=OP.max)
                        T = ep.tile([128, GB, d_out], bf16, tag="T")
                        for b in range(GB):
                            nc.vector.tensor_mul(out=T[:, b, :], in0=Lx[:, b, :],
                                                 in1=attr_sb[li][:])
                        sc = ep.tile([128, GB, heads], f32, tag="sc")
                        for h in range(heads):
                            nc.vector.reduce_sum(out=sc[:, :, h],
                                                 in_=T[:, :, h * cdim:(h + 1) * cdim],
                                                 axis=AX)
                        esc = ep.tile([128, GB, heads], bf16, tag="esc")
                        nc.scalar.activation(out=esc[:], in_=sc[:], func=ACT.Exp)
                        wh = ep.tile([128, GB, heads * SEG_MAX], bf16, tag="wh")
                        for b in range(GB):
                            pslice = pm_sb[:, (GB * g + b) * SEG_MAX:(GB * g + b + 1) * SEG_MAX]
                            for h in range(heads):
                                nc.vector.tensor_mul(
                                    out=wh[:, b, h * SEG_MAX:(h + 1) * SEG_MAX],
                                    in0=pslice,
                                    in1=esc[:, b, h:h + 1].to_broadcast([128, SEG_MAX]))
                        pso = eps.tile([128, ((raw_w + 511) // 512) * 512], f32,
                                       space="PSUM", tag="pso")
                        for b in range(GB):
                            boff = SEG_MAX * b  # psum base partitions: 32-granular
                            pslice = pm_sb[:, (GB * g + b) * SEG_MAX:(GB * g + b + 1) * SEG_MAX]
                            nc.tensor.matmul(pso[boff:boff + SEG_MAX, den_col:den_col + heads],
                                             lhsT=pslice, rhs=esc[:, b, :],
                                             start=True, stop=True,
                                             tile_position=(0, boff))
                            for h in range(heads):
                                nc.tensor.matmul(
                                    pso[boff:boff + SEG_MAX, h * cdim:(h + 1) * cdim],
                                    lhsT=wh[:, b, h * SEG_MAX:(h + 1) * SEG_MAX],
                                    rhs=AL[:, b, h * cdim:(h + 1) * cdim],
                                    start=True, stop=True,
                                    tile_position=(0, boff))
                        stage = ep.tile([128, 1, raw_w], f32, tag="stage")
                        nc.vector.memset(stage[:, 0, den_col + heads:], 0.0)
                        nc.vector.tensor_copy(out=stage[:, 0, :den_col + heads],
                                              in_=pso[:, :den_col + heads])
                        nc.gpsimd.dma_scatter_add(
                            out_ap=cfg["hraw"][:], in_ap=stage[:],
                            idxs_ap=sc_sb[:, GB * SEG_MAX // 16 * g:GB * SEG_MAX // 16 * (g + 1)],
                            num_idxs=GB * SEG_MAX, num_idxs_reg=GB * SEG_MAX,
                            elem_size=raw_w)

                # ---- epilogue ----
                if PHASES < base + 3:
                    break
                with tc.tile_pool(name=f"epi{l}", bufs=3) as qp, \
                     tc.tile_pool(name=f"epips{l}", bufs=2, space="PSUM") as qps:
                    for t in range(NT):
                        rn, r0 = RN[t], t * 128
                        hr = qp.tile([128, raw_w], f32, tag="hr")
                        nc.sync.dma_start(out=hr[:rn, :], in_=cfg["hraw"][r0:r0 + rn, :])
                        rd = qp.tile([128, heads], f32, tag="rd")
                        nc.vector.reciprocal(out=rd[:rn, :],
                                             in_=hr[:rn, den_col:den_col + heads])
                        y = qp.tile([128, d_out], f32, tag="y")
                        for h in range(heads):
                            nc.vector.scalar_tensor_tensor(
                                out=y[:rn, h * cdim:(h + 1) * cdim],
                                in0=hr[:rn, h * cdim:(h + 1) * cdim],
                                scalar=rd[:rn, h:h + 1], op0=OP.mult,
                                in1=biasr_sb[li][:rn, h * cdim:(h + 1) * cdim], op1=OP.add)
                        if l < 3:
                            # ELU -> bf16 h_out; transpose into hT blocked
                            mn = qp.tile([128, d_out], f32, tag="mn")
                            nc.vector.tensor_scalar_min(out=mn[:rn, :], in0=y[:rn, :],
                                                        scalar1=0.0)
                            ex = qp.tile([128, d_out], f32, tag="ex")
                            nc.scalar.activation(out=ex[:rn, :], in_=mn[:rn, :], func=ACT.Exp)
                            hb = qp.tile([128, d_out], bf16, tag="hb")
                            nc.vector.scalar_tensor_tensor(out=hb[:rn, :], in0=ex[:rn, :],
                                                           scalar=-1.0, op0=OP.add,
                                                           in1=y[:rn, :], op1=OP.max)
                            FB = d_out // 128
                            psT = qps.tile([128, FB, 128], bf16, space="PSUM", tag="psT")
                            for f in range(FB):
                                nc.tensor.transpose(out=psT[:, f, :rn],
                                                    in_=hb[:rn, f * 128:(f + 1) * 128],
                                                    identity=ident_sb[:rn, :rn])
                            hTsb = qp.tile([128, FB, 128], bf16, tag="hTsb")
                            nc.vector.tensor_copy(out=hTsb[:, :, :rn], in_=psT[:, :, :rn])
                            nc.sync.dma_start(
                                out=cfg["hT"][:, t, :, :rn].rearrange("k p n -> p k n"),
                                in_=hTsb[:, :, :rn])
                        else:
                            # leaky(h3) + h_loc, then logits
                            l3 = qp.tile([128, C], f32, tag="l3")
                            nc.vector.scalar_tensor_tensor(out=l3[:rn, :], in0=y[:rn, :],
                                                           scalar=NEG, op0=OP.mult,
                                                           in1=y[:rn, :], op1=OP.max)
                            hl = qp.tile([128, C], bf16, tag="hl")
                            nc.sync.dma_start(out=hl[:rn, :], in_=h_loc[r0:r0 + rn, :])
                            o = qp.tile([128, C], f32, tag="o")
                            nc.vector.tensor_add(out=o[:rn, :], in0=l3[:rn, :], in1=hl[:rn, :])
                            junk = qp.tile([128, C], f32, tag="junk")
                            nc.vector.tensor_mul(out=junk[:rn, :], in0=o[:rn, :],
                                                 in1=wout_sb[:rn, :])
                            acc = qp.tile([128, 1], f32, tag="acc")
                            nc.vector.reduce_sum(out=acc[:rn, :], in_=junk[:rn, :],
                                                 axis=AX)
                            nc.vector.tensor_scalar_add(out=logit_sb[:rn, t:t + 1],
                                                        in0=acc[:rn, :],
                                                        scalar1=bout_sb[:rn, :])
                prev_hT = cfg["hT"]

            loopctx.__exit__(None, None, None)

            # write logits out
            if PHASES < 13:
                nc.vector.memset(logit_sb[:], 0.0)
            full = (NPC // 128) * 128
            nc.sync.dma_start(
                out=logits[0:full].rearrange("(t p) -> p t", p=128),
                in_=logit_sb[:, 0:NPC // 128])
            if NPC % 128:
                nc.sync.dma_start(out=logits[full:NPC, None],
                                  in_=logit_sb[:NPC - full, NT - 1:NT])

    nc.compile()
    return nc


# ---------------------------------------------------------------------------
# host wrapper
# ---------------------------------------------------------------------------

_CACHE = {}


def _rep(v, rows=128):
    v = np.asarray(v, np.float32).reshape(1, -1)
    return np.tile(v, (rows, 1))


def _prepare(inputs):
    src = np.asarray(inputs["src"])
    dst = np.asarray(inputs["dst"])
    per_core, NB = _preprocess(src, dst)

    x = np.asarray(inputs["x"], np.float32)
    shared = {
        "w_in": np.asarray(inputs["W_in"]).astype(BF),
        "b_in": np.asarray(inputs["b_in"]).reshape(1, -1).astype(BF),
        "woutrep": _rep(np.asarray(inputs["W_out"]).reshape(-1)),
        "boutrep": np.full((128, 1), float(np.asarray(inputs["b_out"]).reshape(-1)[0]),
                           np.float32),
        "identbf": np.eye(128, dtype=np.float32).astype(BF),
        "onesbf": np.ones((1, 128), np.float32).astype(BF),
    }
    for li, l in enumerate((1, 2, 3)):
        shared[f"wl{l}"] = np.asarray(inputs[f"Wl{l}"]).astype(BF)
        shared[f"wr{l}"] = np.asarray(inputs[f"Wr{l}"]).astype(BF)
        shared[f"bl{l}"] = np.asarray(inputs[f"bl{l}"]).reshape(1, -1).astype(BF)
        shared[f"br{l}"] = np.asarray(inputs[f"br{l}"]).reshape(1, -1).astype(BF)
        att = np.asarray(inputs[f"att{l}"]).reshape(-1)      # [heads*cdim]
        shared[f"attrep{l}"] = _rep(att).astype(BF)
        shared[f"biasrep{l}"] = _rep(np.asarray(inputs[f"bias{l}"]).reshape(-1))

    in_maps = []
    for c in range(NCORES):
        m = dict(shared)
        m["x_loc"] = x[c * NPC:(c + 1) * NPC].astype(BF)
        m.update(per_core[c])
        in_maps.append(m)
    return NB, in_maps


def kernel(**inputs):
    import concourse.bass_utils as bass_utils

    NB, in_maps = _prepare(inputs)
    if NB not in _CACHE:
        _CACHE[NB] = _build(NB)
    nc = _CACHE[NB]

    res = bass_utils.run_bass_kernel_spmd(nc, in_maps, core_ids=list(range(NCORES)))
    out = np.concatenate([res.results[c]["logits"] for c in range(NCORES)])
    return out.astype(np.float32)
